# revision 5
# baseline (speedup 1.0000x reference)
"""GATv2 (2-layer, N=50000, E=800000) on 8 Trainium2 NeuronCores.

Strategy (self-contained; shapes hardcoded for nn_GATUnit_34067680592302):
  - Nodes partitioned across 8 cores (6250 each, padded to 6272 = 49 blocks
    of 128). Edges (incl. self-loops) assigned by destination node and sorted
    by destination, so scatter-softmax / segment-sum stay core-local.
  - Per layer, every core holds the full "source transform" table
    xl = x @ Wl in its DRAM (layer 1: computed locally from replicated x;
    layer 2: local h shard transformed then AllGather'ed), and gathers
    xl[src] rows per edge with indirect DMA.
  - Per 128-node block, attention + weighted aggregation accumulate in PSUM
    via selection-matrix matmuls; a final reciprocal-scale epilogue divides
    by the softmax denominators (max-subtraction is skipped: |s| <= ~10 so
    exp() is safe in fp32).
"""
import sys
sys.path.insert(0, "/opt/trn_rl_repo")

import numpy as np

NEG = 0.2


class Cfg:
    def __init__(self, N=50000, E=800000, ncores=8, nloc=6250, F=128):
        assert N == ncores * nloc
        self.N, self.E, self.ncores, self.nloc, self.F = N, E, ncores, nloc, F
        self.nblk = -(-nloc // 128)          # blocks of 128 nodes per core
        self.nlocp = self.nblk * 128         # padded local nodes
        self.npad = ncores * self.nlocp      # padded global nodes
        assert self.npad % 512 == 0


CFG = Cfg()


# --------------------------------------------------------------------------
# Host-side preprocessing
# --------------------------------------------------------------------------

def host_prep(x, edge_index, Wl1, Wr1, att1, b1, Wl2, Wr2, att2, b2, cfg):
    N, E, NC, NLOC = cfg.N, cfg.E, cfg.ncores, cfg.nloc
    NBLK, NLOCP, NPAD, F = cfg.nblk, cfg.nlocp, cfg.npad, cfg.F

    src0 = np.asarray(edge_index[0]).astype(np.int64)
    dst0 = np.asarray(edge_index[1]).astype(np.int64)
    loops = np.arange(N, dtype=np.int64)
    SRC = np.concatenate([src0, loops])
    DST = np.concatenate([dst0, loops])
    shard = DST // NLOC
    src_g = ((SRC // NLOC) * NLOCP + (SRC % NLOC)).astype(np.int32)
    dst_loc = (DST - shard * NLOC).astype(np.int64)

    per_core = []
    cnts = np.zeros((NC, NBLK), dtype=np.int64)
    for c in range(NC):
        sel = shard == c
        sg, dl = src_g[sel], dst_loc[sel]
        order = np.argsort(dl, kind="stable")
        sg, dl = sg[order], dl[order]
        per_core.append((sg, dl))
        cnts[c] = np.bincount(dl // 128, minlength=NBLK)

    pcnt = (-(-cnts.max(axis=0) // 128) * 128).astype(np.int64)  # per-block padded
    offs = np.concatenate([[0], np.cumsum(pcnt)])
    NTOT = int(offs[-1])
    NT = NTOT // 128
    tiles_b = (pcnt // 128).astype(np.int64)
    st_b = (-(-tiles_b // 4)).astype(np.int64)
    NST = int(st_b.sum())

    core_arrays = []
    for c in range(NC):
        sg, dl = per_core[c]
        cb = np.concatenate([[0], np.cumsum(cnts[c])])
        src_arr = np.zeros(NTOT, dtype=np.int32)
        dst_arr = np.full(NTOT, -1.0, dtype=np.float32)
        for b in range(NBLK):
            n = int(cnts[c][b])
            o = int(offs[b])
            src_arr[o:o + n] = sg[cb[b]:cb[b] + n]
            dst_arr[o:o + n] = dl[cb[b]:cb[b] + n] - 128 * b
        srccol = np.ascontiguousarray(src_arr.reshape(NT, 128).T)
        dstcol = np.ascontiguousarray(dst_arr.reshape(NT, 128).T)
        dstrow = np.full((NST, 512), -1.0, dtype=np.float32)
        st = 0
        for b in range(NBLK):
            for g in range(int(st_b[b])):
                lo = int(offs[b]) + 512 * g
                hi = min(lo + 512, int(offs[b]) + int(pcnt[b]))
                dstrow[st, :hi - lo] = dst_arr[lo:hi]
                st += 1
        core_arrays.append(dict(srccol=srccol, dstcol=dstcol, dstrow=dstrow))

    # node features, transposed + padded: xT[f, g] with g = s*NLOCP + j
    x = np.asarray(x, dtype=np.float32)
    xpad = np.zeros((NPAD, F), dtype=np.float32)
    for s in range(NC):
        xpad[s * NLOCP:s * NLOCP + NLOC] = x[s * NLOC:(s + 1) * NLOC]
    xTfull = np.ascontiguousarray(xpad.T)

    H1 = att1.shape[0]
    C1 = att1.shape[1]
    att1m = np.zeros((128, H1), dtype=np.float32)
    for h in range(H1):
        att1m[h * C1:(h + 1) * C1, h] = att1[h]
    att2m = np.zeros((128, 1), dtype=np.float32)
    att2m[:att2.shape[1], 0] = att2[0]

    iota = np.arange(128, dtype=np.float32)
    padmat = (np.arange(NLOCP) >= NLOC).astype(np.float32).reshape(1, NLOCP)

    import ml_dtypes
    bf16 = ml_dtypes.bfloat16
    shared = dict(
        xTfull=xTfull,
        Wl1=np.asarray(Wl1, np.float32), Wr1=np.asarray(Wr1, np.float32),
        Wl2=np.asarray(Wl2, np.float32), Wr2=np.asarray(Wr2, np.float32),
        att1m=att1m, att2m=att2m,
        bias1r=np.tile(np.asarray(b1, np.float32), (128, 1)),
        bias2r=np.tile(np.asarray(b2, np.float32), (128, 1)),
        id128=np.eye(128, dtype=np.float32),
        iorow=np.tile(iota, (128, 1)),
        iocol=iota.reshape(128, 1).copy(),
        iocolb=iota.reshape(128, 1).astype(bf16),
        padmat=padmat,
        e01=np.concatenate([np.ones(H1, np.float32),
                            np.zeros(F, np.float32)]).reshape(1, H1 + F),
        e02=np.concatenate([np.ones(1, np.float32),
                            np.zeros(F, np.float32)]).reshape(1, 1 + F),
    )
    in_maps = []
    for c in range(NC):
        m = dict(shared)
        m["xTloc"] = np.ascontiguousarray(xTfull[:, c * NLOCP:(c + 1) * NLOCP])
        m.update(core_arrays[c])
        m["dstrow"] = m["dstrow"].astype(bf16)
        in_maps.append(m)
    meta = dict(pcnt=pcnt, tiles_b=tiles_b, st_b=st_b, NT=NT, NST=NST, H1=H1)
    return in_maps, meta


# --------------------------------------------------------------------------
# Device program
# --------------------------------------------------------------------------

def build_nc(cfg, meta, profile_nocc=False):
    import concourse.bacc as bacc
    import concourse.tile as tile
    from concourse import mybir
    from concourse.bass import IndirectOffsetOnAxis

    f32 = mybir.dt.float32
    bf16 = mybir.dt.bfloat16
    i32 = mybir.dt.int32
    AF = mybir.ActivationFunctionType
    OP = mybir.AluOpType

    NC, F = cfg.ncores, cfg.F
    NBLK, NLOCP, NPAD = cfg.nblk, cfg.nlocp, cfg.npad
    NT, NST, H1 = meta["NT"], meta["NST"], meta["H1"]
    tiles_b, st_b = meta["tiles_b"], meta["st_b"]

    nc = bacc.Bacc("TRN2", target_bir_lowering=False)

    din = {}
    def ein(name, shape, dt=f32):
        din[name] = nc.dram_tensor(name, shape, dt, kind="ExternalInput")
        return din[name]

    d_xTfull = ein("xTfull", [128, NPAD])
    d_xTloc = ein("xTloc", [128, NLOCP])
    d_Wl1, d_Wr1 = ein("Wl1", [128, 128]), ein("Wr1", [128, 128])
    d_Wl2, d_Wr2 = ein("Wl2", [128, 128]), ein("Wr2", [128, 128])
    d_att1, d_att2 = ein("att1m", [128, H1]), ein("att2m", [128, 1])
    d_b1r, d_b2r = ein("bias1r", [128, F]), ein("bias2r", [128, F])
    d_id = ein("id128", [128, 128])
    d_iorow, d_iocol = ein("iorow", [128, 128]), ein("iocol", [128, 1])
    d_iocolb = ein("iocolb", [128, 1], bf16)
    d_padm = ein("padmat", [1, NLOCP])
    d_e01, d_e02 = ein("e01", [1, H1 + F]), ein("e02", [1, 1 + F])
    d_srccol = ein("srccol", [128, NT], i32)
    d_dstcol = ein("dstcol", [128, NT])
    d_dstrow = ein("dstrow", [NST, 512], bf16)

    d_out = nc.dram_tensor("outloc", [NLOCP, F], f32, kind="ExternalOutput")

    d_xl1 = nc.dram_tensor("xl1tab", [NPAD, F], f32)
    d_xl2 = nc.dram_tensor("xl2tab", [NPAD, F], f32)
    d_hTloc = nc.dram_tensor("hTloc", [128, NLOCP], f32)
    d_hTfull = nc.dram_tensor("hTfull", [NC * 128, NLOCP], f32,
                              addr_space="Shared")

    with tile.TileContext(nc) as tc:
        with tc.tile_pool(name="const", bufs=1) as cp:
            Wl1_sb = cp.tile_from(d_Wl1[:, :])
            Wr1_sb = cp.tile_from(d_Wr1[:, :])
            Wl2_sb = cp.tile_from(d_Wl2[:, :])
            Wr2_sb = cp.tile_from(d_Wr2[:, :])
            att1_sb = cp.tile_from(d_att1[:, :])
            att2_sb = cp.tile_from(d_att2[:, :])
            b1_sb = cp.tile_from(d_b1r[:, :])
            b2_sb = cp.tile_from(d_b2r[:, :])
            id_sb = cp.tile_from(d_id[:, :])
            iorow_sb = cp.tile_from(d_iorow[:, :])
            iocol_sb = cp.tile_from(d_iocol[:, :])
            iocolb_sb = cp.tile_from(d_iocolb[:, :])
            padm_sb = cp.tile_from(d_padm[:, :])
            e01_sb = cp.tile_from(d_e01[:, :])
            e02_sb = cp.tile_from(d_e02[:, :])
            src_sb = cp.tile_from(d_srccol[:, :])
            dstc_sb = cp.tile_from(d_dstcol[:, :])
            xr1h_sb = cp.tile([128, NLOCP], bf16)
            xr1l_sb = cp.tile([128, NLOCP], bf16)
            xr2h_sb = cp.tile([128, NLOCP], bf16)
            xr2l_sb = cp.tile([128, NLOCP], bf16)

            # ---------------- phase A: layer-1 tables ----------------
            with (
                tc.tile_pool(name="tabs", bufs=4) as tp,
                tc.tile_pool(name="tabp", bufs=2, space="PSUM") as tpp,
            ):
                for t in range(NBLK):  # xr1 for local nodes
                    xt = tp.tile([128, 128], f32, tag="xt")
                    nc.sync.dma_start(out=xt[:], in_=d_xTloc[:, t * 128:(t + 1) * 128])
                    ps = tpp.tile([128, 128], f32, tag="psx")
                    nc.tensor.matmul(out=ps[:], lhsT=xt[:], rhs=Wr1_sb[:],
                                     start=True, stop=True)
                    nc.vector.tensor_copy(xr1h_sb[:, t * 128:(t + 1) * 128], ps[:])
                    nc.vector.tensor_tensor(
                        out=xr1l_sb[:, t * 128:(t + 1) * 128], in0=ps[:],
                        in1=xr1h_sb[:, t * 128:(t + 1) * 128], op=OP.subtract)
                for t4 in range(NPAD // 512):  # full xl1 table, 4 tiles/bank
                    ps4 = tpp.tile([128, 512], f32, tag="ps4")
                    for j in range(4):
                        t = 4 * t4 + j
                        xt = tp.tile([128, 128], f32, tag="xt")
                        nc.sync.dma_start(out=xt[:],
                                          in_=d_xTfull[:, t * 128:(t + 1) * 128])
                        nc.tensor.matmul(out=ps4[:, j * 128:(j + 1) * 128],
                                         lhsT=xt[:], rhs=Wl1_sb[:],
                                         start=(j == 0), stop=(j == 3),
                                         skip_group_check=True)
                    stg = tp.tile([128, 512], f32, tag="stg")
                    nc.vector.tensor_copy(stg[:], ps4[:])
                    nc.sync.dma_start(
                        out=d_xl1[t4 * 512:(t4 + 1) * 512, :]
                            .rearrange("(t p) f -> p t f", p=128),
                        in_=stg[:].rearrange("p (t f) -> p t f", t=4),
                    )

            # ---------------- phase B: layer-1 edges ----------------
            def edge_layer(H, d_xltab, xr_hi, xr_lo, att_sb, e0_sb, bias_sb, epilogue):
                with (
                    tc.tile_pool(name="ep", bufs=3) as wp,
                    tc.tile_pool(name="epp", bufs=2, space="PSUM") as pp,
                    tc.tile_pool(name="epp1", bufs=1, space="PSUM") as pp1,
                ):
                    st = 0
                    t0 = 0
                    for b in range(NBLK):
                        b_acc = pp.tile([128, H + F], f32, tag="b_acc")
                        nc.tensor.matmul(out=b_acc[:],
                                         lhsT=padm_sb[0:1, b * 128:(b + 1) * 128],
                                         rhs=e0_sb[0:1, 0:H + F],
                                         start=True, stop=False,
                                         skip_group_check=True)
                        ntb = int(tiles_b[b])
                        for g in range(int(st_b[b])):
                            sz = min(4, ntb - 4 * g)
                            E1 = sz * 128
                            xl_g = wp.tile([128, 4 * F], f32, tag="xl_g")
                            for j in range(sz):
                                nc.gpsimd.indirect_dma_start(
                                    out=xl_g[:, j * F:(j + 1) * F],
                                    out_offset=None,
                                    in_=d_xltab[:, :],
                                    in_offset=IndirectOffsetOnAxis(
                                        ap=src_sb[:, t0 + 4 * g + j:t0 + 4 * g + j + 1],
                                        axis=0),
                                )
                            b_et = pp.tile([128, 512], f32, tag="b_et")
                            for j in range(sz):
                                nc.tensor.matmul(
                                    out=b_et[:, j * 128:(j + 1) * 128],
                                    lhsT=xl_g[:, j * F:(j + 1) * F],
                                    rhs=id_sb[:], is_transpose=True,
                                    start=(j == 0), stop=False,
                                    skip_group_check=True)
                            drst = wp.tile([1, 512], bf16, tag="drst")
                            nc.sync.dma_start(out=drst[0:1, :],
                                              in_=d_dstrow[st:st + 1, :])
                            dstrep = wp.tile([128, 512], bf16, tag="dstrep")
                            nc.gpsimd.partition_broadcast(
                                dstrep[:, 0:E1], drst[0:1, 0:E1])
                            m2 = wp.tile([128, 512], bf16, tag="m2")
                            nc.vector.tensor_tensor(
                                out=m2[:, 0:E1], in0=dstrep[:, 0:E1],
                                in1=iocolb_sb[:].to_broadcast([128, E1]),
                                op=OP.is_equal)
                            nc.tensor.matmul(
                                out=b_et[:, 0:E1],
                                lhsT=xr_hi[:, b * 128:(b + 1) * 128],
                                rhs=m2[:, 0:E1],
                                start=False, stop=False, skip_group_check=True)
                            nc.tensor.matmul(
                                out=b_et[:, 0:E1],
                                lhsT=xr_lo[:, b * 128:(b + 1) * 128],
                                rhs=m2[:, 0:E1],
                                start=False, stop=True, skip_group_check=True)
                            relu = wp.tile([128, 512], f32, tag="relu")
                            nc.scalar.activation(out=relu[:, 0:E1],
                                                 in_=b_et[:, 0:E1],
                                                 func=AF.Relu, scale=1.0 - NEG)
                            lrel = wp.tile([128, 512], f32, tag="lrel")
                            nc.vector.scalar_tensor_tensor(
                                out=lrel[:, 0:E1], in0=b_et[:, 0:E1],
                                scalar=NEG, in1=relu[:, 0:E1],
                                op0=OP.mult, op1=OP.add)
                            m = wp.tile([128, 512], f32, tag="m")
                            nc.vector.tensor_tensor(
                                out=m[:, 0:E1].rearrange("p (t n) -> p t n", t=sz),
                                in0=dstc_sb[:, t0 + 4 * g:t0 + 4 * g + sz]
                                    .unsqueeze(2).to_broadcast([128, sz, 128]),
                                in1=iorow_sb[:].unsqueeze(1)
                                    .to_broadcast([128, sz, 128]),
                                op=OP.is_equal)
                            b_s = pp.tile([128, 4 * H], f32, tag="b_s")
                            for j in range(sz):
                                nc.tensor.matmul(
                                    out=b_s[:, j * H:(j + 1) * H],
                                    lhsT=lrel[:, j * 128:(j + 1) * 128],
                                    rhs=att_sb[:, 0:H],
                                    start=(j == 0), stop=(j == sz - 1),
                                    skip_group_check=True)
                            w_sb = wp.tile([128, 4 * (H + F)], f32, tag="w_sb")
                            nc.scalar.activation(
                                out=w_sb[:].rearrange("p (t x) -> p t x",
                                                      t=4)[:, 0:sz, 0:H],
                                in_=b_s[:, 0:sz * H]
                                    .rearrange("p (t h) -> p t h", t=sz),
                                func=AF.Exp)
                            nc.vector.tensor_tensor(
                                out=w_sb[:].rearrange("p (t x) -> p t x",
                                                      t=4)[:, 0:sz, H:H + F]
                                    .rearrange("p t (h c) -> p t h c", h=H),
                                in0=xl_g[:, 0:sz * F]
                                    .rearrange("p (t h c) -> p t h c", t=sz, h=H),
                                in1=w_sb[:].rearrange("p (t x) -> p t x",
                                                      t=4)[:, 0:sz, 0:H]
                                    .unsqueeze(3).to_broadcast([128, sz, H, F // H]),
                                op=OP.mult)
                            for j in range(sz):
                                nc.tensor.matmul(
                                    out=b_acc[:],
                                    lhsT=m[:, j * 128:(j + 1) * 128],
                                    rhs=w_sb[:, j * (H + F):(j + 1) * (H + F)],
                                    start=False,
                                    stop=(g == int(st_b[b]) - 1 and j == sz - 1),
                                    skip_group_check=True)
                            st += 1
                        t0 += ntb
                        # epilogue: divide by denominators, add bias
                        recip = wp.tile([128, H], f32, tag="recip")
                        nc.vector.reciprocal(recip[:], b_acc[:, 0:H])
                        outb = wp.tile([128, F], f32, tag="outb")
                        C = F // H
                        for h in range(H):
                            nc.vector.tensor_scalar_mul(
                                outb[:, h * C:(h + 1) * C],
                                b_acc[:, H + h * C:H + (h + 1) * C],
                                recip[:, h:h + 1])
                        nc.vector.tensor_tensor(out=outb[:], in0=outb[:],
                                                in1=bias_sb[:], op=OP.add)
                        epilogue(b, outb, wp, pp1)

            def epi1(b, outb, wp, pp1):
                ps_h = pp1.tile([128, 128], f32, tag="ps_h")
                nc.tensor.matmul(out=ps_h[:], lhsT=outb[:], rhs=id_sb[:],
                                 is_transpose=True, start=True, stop=True)
                hT = wp.tile([128, 128], f32, tag="hT")
                nc.vector.tensor_copy(hT[:], ps_h[:])
                ps_x = pp1.tile([128, 128], f32, tag="ps_x2")
                nc.tensor.matmul(out=ps_x[:], lhsT=hT[:], rhs=Wr2_sb[:],
                                 start=True, stop=True)
                nc.vector.tensor_copy(xr2h_sb[:, b * 128:(b + 1) * 128], ps_x[:])
                nc.vector.tensor_tensor(
                    out=xr2l_sb[:, b * 128:(b + 1) * 128], in0=ps_x[:],
                    in1=xr2h_sb[:, b * 128:(b + 1) * 128], op=OP.subtract)
                nc.sync.dma_start(out=d_hTloc[:, b * 128:(b + 1) * 128],
                                  in_=hT[:])

            edge_layer(H1, d_xl1, xr1h_sb, xr1l_sb, att1_sb, e01_sb, b1_sb, epi1)

            # ---------------- phase C: AllGather h^T ----------------
            if profile_nocc:
                # timeline-sim variant: stand in for the collective with DMAs
                # of equivalent local traffic (write 1 shard, read NC back)
                for s in range(NC):
                    nc.sync.dma_start(out=d_hTfull[s * 128:(s + 1) * 128, :],
                                      in_=d_hTloc[:, :])
            else:
                nc.gpsimd.collective_compute(
                    "AllGather",
                    mybir.AluOpType.bypass,
                    replica_groups=[list(range(NC))],
                    ins=[d_hTloc[:, :]],
                    outs=[d_hTfull[:, :]],
                )

            # ---------------- phase D: layer-2 xl table ----------------
            with (
                tc.tile_pool(name="tabs2", bufs=4) as tp,
                tc.tile_pool(name="tabp2", bufs=2, space="PSUM") as tpp,
            ):
                for t4 in range(NPAD // 512):
                    ps4 = tpp.tile([128, 512], f32, tag="ps4")
                    for j in range(4):
                        t = 4 * t4 + j
                        s, tb = t // NBLK, t % NBLK
                        ht = tp.tile([128, 128], f32, tag="ht")
                        nc.sync.dma_start(
                            out=ht[:],
                            in_=d_hTfull[s * 128:(s + 1) * 128,
                                         tb * 128:(tb + 1) * 128])
                        nc.tensor.matmul(out=ps4[:, j * 128:(j + 1) * 128],
                                         lhsT=ht[:], rhs=Wl2_sb[:],
                                         start=(j == 0), stop=(j == 3),
                                         skip_group_check=True)
                    stg = tp.tile([128, 512], f32, tag="stg")
                    nc.vector.tensor_copy(stg[:], ps4[:])
                    nc.sync.dma_start(
                        out=d_xl2[t4 * 512:(t4 + 1) * 512, :]
                            .rearrange("(t p) f -> p t f", p=128),
                        in_=stg[:].rearrange("p (t f) -> p t f", t=4),
                    )

            # ---------------- phase E: layer-2 edges ----------------
            def epi2(b, outb, wp, pp1):
                nc.sync.dma_start(out=d_out[b * 128:(b + 1) * 128, :],
                                  in_=outb[:])

            edge_layer(1, d_xl2, xr2h_sb, xr2l_sb, att2_sb, e02_sb, b2_sb, epi2)

    nc.compile()
    return nc


# --------------------------------------------------------------------------
# Entry point
# --------------------------------------------------------------------------

def kernel(x, edge_index, edge_attr, Wl1, Wr1, att1, b1, Wl2, Wr2, att2, b2,
           cfg=None, _want_results=False):
    from concourse.bass_utils import run_bass_kernel_spmd

    cfg = cfg or CFG
    in_maps, meta = host_prep(x, edge_index, Wl1, Wr1, att1, b1,
                              Wl2, Wr2, att2, b2, cfg)
    nc = build_nc(cfg, meta)
    res = run_bass_kernel_spmd(nc, in_maps, core_ids=list(range(cfg.ncores)))
    out = np.empty((cfg.N, cfg.F), dtype=np.float32)
    for c in range(cfg.ncores):
        out[c * cfg.nloc:(c + 1) * cfg.nloc] = \
            res.results[c]["outloc"][:cfg.nloc]
    if _want_results:
        return out, res
    return out
